# revision 3
# baseline (speedup 1.0000x reference)
"""Trainium2 Bass kernel for Mixtral-style MoE (8 experts, top-2, SwiGLU).

Strategy: data-parallel over tokens across 8 NeuronCores (1024 tokens/core),
weights replicated. Per core:
  - router logits in fp32 on PE (top-2 selection must match the fp32 reference)
  - top-2 + renormalized weights via sigmoid(l1-l2) identity, all on DVE/ACT
  - expert FFN (w1/w3/w2) in bf16 with fp32 PSUM accumulation, computed
    densely for all 8 experts; per-token weights applied to the expert output
    via a ones-column broadcast matmul, accumulated in SBUF fp32.
No collectives: host concatenates per-core outputs.

Layouts (host-prepared, per core):
  xt_f32 / xt_bf16 : [H, Tc]  (tokens on the free dim; contraction dim H on
                     partitions for all GEMMs)
  gate_t           : [H, E] fp32
  w1t/w3t          : [E, I/128, 128, H]  blocked so lhsT tile (k) is a column
                     slice of a contiguous [128, H] slab; slab row p, col
                     k*128+c holds w1[e, i*128+c, k*128+p] (i.e. w1[e].T)
  w2t              : [E, H/128, 128, I]  same idea for w2[e].T
  out              : [H, Tc] fp32 (host transposes back)
"""

import numpy as np
import ml_dtypes

import concourse.bass as bass
import concourse.mybir as mybir
import concourse.tile as tile
from concourse import bacc
from concourse.masks import make_identity

P = 128
FP32 = mybir.dt.float32
BF16 = mybir.dt.bfloat16

# Full-problem constants
N_CORES = 8
NUM_TOKENS = 8192
HIDDEN = 2048
INTER = 4096
EXPERTS = 8
T_CORE = NUM_TOKENS // N_CORES


def build_program(t_core=T_CORE, h=HIDDEN, i_sz=INTER, e_num=EXPERTS,
                  t_chunk=512):
    assert t_core % t_chunk == 0
    kt = h // P          # k tiles (contraction for GEMM1 / router)
    it = i_sz // P       # i tiles
    ht = h // P          # output h tiles
    tt_n = t_chunk // P  # token tiles per chunk (router granularity)
    n_chunks = t_core // t_chunk

    nc = bacc.Bacc("TRN2", target_bir_lowering=False, debug=False)

    xt_f32 = nc.dram_tensor("xt_f32", [h, t_core], FP32, kind="ExternalInput").ap()
    xt_bf16 = nc.dram_tensor("xt_bf16", [h, t_core], BF16, kind="ExternalInput").ap()
    gate_t = nc.dram_tensor("gate_t", [h, e_num], FP32, kind="ExternalInput").ap()
    w1t = nc.dram_tensor("w1t", [e_num, it, P, h], BF16, kind="ExternalInput").ap()
    w3t = nc.dram_tensor("w3t", [e_num, it, P, h], BF16, kind="ExternalInput").ap()
    w2t = nc.dram_tensor("w2t", [e_num, ht, P, i_sz], BF16, kind="ExternalInput").ap()
    out_d = nc.dram_tensor("out", [h, t_core], FP32, kind="ExternalOutput").ap()

    with tile.TileContext(nc) as tc:
        with (
            tc.tile_pool(name="const", bufs=1) as const_pool,
            tc.tile_pool(name="resident", bufs=1) as res_pool,
            tc.tile_pool(name="stream", bufs=3) as stream_pool,
            tc.tile_pool(name="w2stream", bufs=2) as w2_pool,
            tc.tile_pool(name="small", bufs=2) as small_pool,
            tc.tile_pool(name="work", bufs=2) as work_pool,
            tc.tile_pool(name="psum", bufs=2, space="PSUM") as psum_pool,
            tc.tile_pool(name="psum_s", bufs=2, space="PSUM") as psum_s,
        ):
            ident = const_pool.tile([P, P], FP32, tag="ident")
            make_identity(nc, ident[:])
            ones1 = const_pool.tile([1, P], FP32, tag="ones1")
            nc.vector.memset(ones1[:], 1.0)

            # gate weights resident
            gt = []
            for k in range(kt):
                g = res_pool.tile([P, e_num], FP32, tag=f"gt{k}")
                nc.sync.dma_start(out=g[:], in_=gate_t[k * P:(k + 1) * P, :])
                gt.append(g)

            for c in range(n_chunks):
                csl = slice(c * t_chunk, (c + 1) * t_chunk)

                # x transposed, bf16, resident for this chunk
                xtb = []
                for k in range(kt):
                    x = res_pool.tile([P, t_chunk], BF16, tag=f"xtb{k}")
                    nc.sync.dma_start(out=x[:], in_=xt_bf16[k * P:(k + 1) * P, csl])
                    xtb.append(x)

                # ---------------- router ----------------
                # weights-by-token rows, one row per expert: [1, e_num*t_chunk]
                wrow = res_pool.tile([1, e_num * t_chunk], FP32, tag="wrow")
                for t in range(tt_n):
                    tsl = slice(c * t_chunk + t * P, c * t_chunk + (t + 1) * P)
                    lg_ps = psum_s.tile([P, e_num], FP32, tag="s")
                    for k in range(kt):
                        xf = small_pool.tile([P, P], FP32, tag="xf")
                        nc.sync.dma_start(out=xf[:], in_=xt_f32[k * P:(k + 1) * P, tsl])
                        nc.tensor.matmul(out=lg_ps[:], lhsT=xf[:], rhs=gt[k][:],
                                         start=(k == 0), stop=(k == kt - 1))
                    l = small_pool.tile([P, e_num], FP32, tag="l")
                    nc.vector.tensor_copy(out=l[:], in_=lg_ps[:])
                    m1 = small_pool.tile([P, 1], FP32, tag="m1")
                    nc.vector.reduce_max(out=m1[:], in_=l[:], axis=mybir.AxisListType.X)
                    mask1 = small_pool.tile([P, e_num], FP32, tag="mask1")
                    nc.vector.tensor_scalar(out=mask1[:], in0=l[:], scalar1=m1[:, :1],
                                            scalar2=None, op0=mybir.AluOpType.is_equal)
                    lm = small_pool.tile([P, e_num], FP32, tag="lm")
                    nc.vector.scalar_tensor_tensor(
                        out=lm[:], in0=mask1[:], scalar=-1e30, in1=l[:],
                        op0=mybir.AluOpType.mult, op1=mybir.AluOpType.add)
                    m2 = small_pool.tile([P, 1], FP32, tag="m2")
                    nc.vector.reduce_max(out=m2[:], in_=lm[:], axis=mybir.AxisListType.X)
                    keep = small_pool.tile([P, e_num], FP32, tag="keep")
                    nc.vector.tensor_scalar(out=keep[:], in0=l[:], scalar1=m2[:, :1],
                                            scalar2=None, op0=mybir.AluOpType.is_ge)
                    mask2 = small_pool.tile([P, e_num], FP32, tag="mask2")
                    nc.vector.tensor_sub(out=mask2[:], in0=keep[:], in1=mask1[:])
                    d = small_pool.tile([P, 1], FP32, tag="d")
                    nc.vector.tensor_sub(out=d[:], in0=m1[:], in1=m2[:])
                    s1 = small_pool.tile([P, 1], FP32, tag="s1")
                    nc.scalar.activation(out=s1[:], in_=d[:],
                                         func=mybir.ActivationFunctionType.Sigmoid)
                    s2 = small_pool.tile([P, 1], FP32, tag="s2")
                    nc.vector.tensor_scalar(out=s2[:], in0=s1[:], scalar1=-1.0,
                                            scalar2=1.0, op0=mybir.AluOpType.mult,
                                            op1=mybir.AluOpType.add)
                    wa = small_pool.tile([P, e_num], FP32, tag="wa")
                    nc.vector.tensor_scalar(out=wa[:], in0=mask1[:], scalar1=s1[:, :1],
                                            scalar2=None, op0=mybir.AluOpType.mult)
                    wfin = small_pool.tile([P, e_num], FP32, tag="wfin")
                    nc.vector.scalar_tensor_tensor(
                        out=wfin[:], in0=mask2[:], scalar=s2[:, :1], in1=wa[:],
                        op0=mybir.AluOpType.mult, op1=mybir.AluOpType.add)
                    # transpose each expert's weight column into the wrow row
                    for e in range(e_num):
                        tp = psum_s.tile([1, P], FP32, tag="s")
                        nc.tensor.transpose(out=tp[:], in_=wfin[:, e:e + 1],
                                            identity=ident[:])
                        nc.vector.tensor_copy(
                            out=wrow[0:1, e * t_chunk + t * P:
                                      e * t_chunk + (t + 1) * P],
                            in_=tp[:])

                # broadcast weight rows to [P, t_chunk] per expert
                wb = []
                for e in range(e_num):
                    wb_ps = psum_s.tile([P, t_chunk], FP32, tag="s")
                    nc.tensor.matmul(
                        out=wb_ps[:], lhsT=ones1[:],
                        rhs=wrow[0:1, e * t_chunk:(e + 1) * t_chunk],
                        start=True, stop=True)
                    wbe = res_pool.tile([P, t_chunk], FP32, tag=f"wb{e}")
                    nc.vector.tensor_copy(out=wbe[:], in_=wb_ps[:])
                    wb.append(wbe)

                # out accumulators
                out_sb = []
                for hh in range(ht):
                    o = res_pool.tile([P, t_chunk], FP32, tag=f"out{hh}")
                    out_sb.append(o)

                # ---------------- experts ----------------
                for e in range(e_num):
                    h_sb = []
                    for i in range(it):
                        w1s = stream_pool.tile([P, h], BF16, tag="w1s")
                        nc.sync.dma_start(out=w1s[:], in_=w1t[e, i])
                        w3s = stream_pool.tile([P, h], BF16, tag="w3s")
                        nc.sync.dma_start(out=w3s[:], in_=w3t[e, i])
                        h1_ps = psum_pool.tile([P, t_chunk], FP32, tag="h1")
                        h3_ps = psum_pool.tile([P, t_chunk], FP32, tag="h3")
                        for k in range(kt):
                            nc.tensor.matmul(out=h1_ps[:],
                                             lhsT=w1s[:, k * P:(k + 1) * P],
                                             rhs=xtb[k][:],
                                             start=(k == 0), stop=(k == kt - 1))
                        for k in range(kt):
                            nc.tensor.matmul(out=h3_ps[:],
                                             lhsT=w3s[:, k * P:(k + 1) * P],
                                             rhs=xtb[k][:],
                                             start=(k == 0), stop=(k == kt - 1))
                        sg = work_pool.tile([P, t_chunk], FP32, tag="sg")
                        nc.scalar.activation(out=sg[:], in_=h1_ps[:],
                                             func=mybir.ActivationFunctionType.Sigmoid)
                        sil = work_pool.tile([P, t_chunk], FP32, tag="sil")
                        nc.vector.tensor_tensor(out=sil[:], in0=sg[:], in1=h1_ps[:],
                                                op=mybir.AluOpType.mult)
                        hcur = res_pool.tile([P, t_chunk], BF16, tag=f"h{i}")
                        nc.vector.tensor_tensor(out=hcur[:], in0=sil[:], in1=h3_ps[:],
                                                op=mybir.AluOpType.mult)
                        h_sb.append(hcur)

                    for hh in range(ht):
                        w2s = w2_pool.tile([P, i_sz], BF16, tag="w2s")
                        nc.sync.dma_start(out=w2s[:], in_=w2t[e, hh])
                        f_ps = psum_pool.tile([P, t_chunk], FP32, tag="f")
                        for i in range(it):
                            nc.tensor.matmul(out=f_ps[:],
                                             lhsT=w2s[:, i * P:(i + 1) * P],
                                             rhs=h_sb[i][:],
                                             start=(i == 0), stop=(i == it - 1))
                        if e == 0:
                            nc.vector.tensor_tensor(out=out_sb[hh][:], in0=f_ps[:],
                                                    in1=wb[e][:],
                                                    op=mybir.AluOpType.mult)
                        else:
                            tmp = work_pool.tile([P, t_chunk], FP32, tag="tmpo")
                            nc.vector.tensor_tensor(out=tmp[:], in0=f_ps[:],
                                                    in1=wb[e][:],
                                                    op=mybir.AluOpType.mult)
                            nc.vector.tensor_tensor(out=out_sb[hh][:],
                                                    in0=out_sb[hh][:], in1=tmp[:],
                                                    op=mybir.AluOpType.add)

                for hh in range(ht):
                    nc.sync.dma_start(out=out_d[hh * P:(hh + 1) * P, csl],
                                      in_=out_sb[hh][:])

    nc.compile()
    return nc


# ---------------------------------------------------------------------------
# host side
# ---------------------------------------------------------------------------

def _block_w1_like(w):
    """[I, H] (already e-sliced, still fp32 or bf16) -> [I/128, 128, H] blocked
    so that slab[i][p, k*128+c] = w.T[k*128+p, i*128+c] = w[i*128+c, k*128+p]."""
    i_sz, h = w.shape
    it, kt = i_sz // P, h // P
    # target: blk[i, p, k*128+c] = w[i*128+c, k*128+p]
    v = w.reshape(it, P, kt, P)            # [i, c, k, p]
    return np.ascontiguousarray(v.transpose(0, 3, 2, 1)).reshape(it, P, h)


def _prep_weights(w1, w3, w2):
    w1b = np.asarray(w1, dtype=ml_dtypes.bfloat16)
    w3b = np.asarray(w3, dtype=ml_dtypes.bfloat16)
    w2b = np.asarray(w2, dtype=ml_dtypes.bfloat16)
    e_num = w1b.shape[0]
    w1t = np.stack([_block_w1_like(w1b[e]) for e in range(e_num)])
    w3t = np.stack([_block_w1_like(w3b[e]) for e in range(e_num)])
    # w2[e] is [H, I]; same blocking with roles of (I,H) swapped
    w2t = np.stack([_block_w1_like(w2b[e]) for e in range(e_num)])
    return w1t, w3t, w2t


_PROG_CACHE = {}


def _get_program():
    key = "full"
    if key not in _PROG_CACHE:
        _PROG_CACHE[key] = build_program()
    return _PROG_CACHE[key]


def kernel(index, hidden_states, gate_w, w1, w3, w2, _trace=False):
    from concourse.bass_utils import run_bass_kernel_spmd

    idx = int(np.asarray(index))
    hs = np.asarray(hidden_states, dtype=np.float32)
    gate = np.asarray(gate_w[idx], dtype=np.float32)        # [E, H]
    w1x, w3x, w2x = w1[idx], w3[idx], w2[idx]

    nc = _get_program()
    w1t, w3t, w2t = _prep_weights(w1x, w3x, w2x)
    gate_t = np.ascontiguousarray(gate.T)                    # [H, E]

    in_maps = []
    for c in range(N_CORES):
        shard = hs[c * T_CORE:(c + 1) * T_CORE]              # [Tc, H]
        xt = np.ascontiguousarray(shard.T)                   # [H, Tc] fp32
        in_maps.append({
            "xt_f32": xt,
            "xt_bf16": np.asarray(xt, dtype=ml_dtypes.bfloat16),
            "gate_t": gate_t,
            "w1t": w1t, "w3t": w3t, "w2t": w2t,
        })

    res = run_bass_kernel_spmd(nc, in_maps, core_ids=list(range(N_CORES)),
                               trace=False)
    outs = [np.asarray(r["out"], dtype=np.float32).T for r in res.results]
    full = np.concatenate(outs, axis=0)
    kernel._last_in_maps = in_maps
    return full
